# revision 3
# baseline (speedup 1.0000x reference)
"""Trainium2 Bass kernel for ConcentratedGATv2Layer.

Contract: kernel(**inputs) takes FULL unsharded inputs
  features  [8, 512, 128] f32
  adjacency [8, 512, 512] int32
  W_src     [4, 128, 64]  f32
  W_dst     [4, 128, 64]  f32
  a         [4, 64, 1]    f32
returns [8, 512, 64] f32.

Sharding: data-parallel over batch; core b computes batch b.

Per-core algorithm (per head h):
  S = f @ W_src[h], D = f @ W_dst[h]
  leaky(x) = 0.2x + 0.8 relu(x), so
  scores/8 = 0.1 * (a . relu(S_i + D_j)) + 0.025*(S@a)_i + 0.025*(D@a)_j
  The relu pass runs on ACT: tiles [128=2 i's x 64 u, 512 j] with per-partition
  bias = the two source rows; the a-contraction runs on PE with a shifted-window
  zero-padded block-diagonal lhsT accumulating 64 pair-matmuls into one PSUM bank.
  The rank-1 linear part + additive adjacency mask (-1e9) are a per-i-block tile M2.
  Top-32 per row via 4x(max8 + match_replace); sparsemax computed exactly on the
  sorted top-32 block; attn = relu(2*sc_masked - (vm+tau)); sharpen via ACT Square
  (+1e-9 bias) with fused row-sum; normalize (and /4 head-mean) on DVE; PE
  transposes attn and accumulates (attn @ S)^T over heads in PSUM.
"""
import numpy as np
from contextlib import ExitStack

import concourse.tile as tile
from concourse import bacc, mybir
from concourse.bass_utils import run_bass_kernel_spmd

dt = mybir.dt
F32 = dt.float32
I32 = dt.int32
AF = mybir.ActivationFunctionType
AL = mybir.AluOpType
AX = mybir.AxisListType

B, N, D, U, H = 8, 512, 128, 64, 4
TOPK = 32
NEG = -1.0e9
REPL = -3.0e38  # match_replace fill, below any real score

_cache = {}


def _build_program():
    nc = bacc.Bacc("TRN2", target_bir_lowering=False, debug=False, num_devices=B)
    feat_d = nc.dram_tensor("feat", [N, D], F32, kind="ExternalInput").ap()
    adj_d = nc.dram_tensor("adj", [N, N], I32, kind="ExternalInput").ap()
    wsrc_d = nc.dram_tensor("wsrc", [H, D, U], F32, kind="ExternalInput").ap()
    wdst_d = nc.dram_tensor("wdst", [H, D, U], F32, kind="ExternalInput").ap()
    abl_d = nc.dram_tensor("ablock", [H, 128, 256], F32, kind="ExternalInput").ap()
    asm_d = nc.dram_tensor("asm", [U, H], F32, kind="ExternalInput").ap()
    kt_d = nc.dram_tensor("ktile", [128, TOPK], F32, kind="ExternalInput").ap()
    out_d = nc.dram_tensor("out", [N, U], F32, kind="ExternalOutput").ap()

    with tile.TileContext(nc) as tc, ExitStack() as ctx:
        const_p = ctx.enter_context(tc.tile_pool(name="const", bufs=1))
        madd_p = ctx.enter_context(tc.tile_pool(name="madd", bufs=4))
        m2_p = ctx.enter_context(tc.tile_pool(name="m2", bufs=8))
        setup_p = ctx.enter_context(tc.tile_pool(name="setup", bufs=2))
        head_p = ctx.enter_context(tc.tile_pool(name="head", bufs=2))
        act_p = ctx.enter_context(tc.tile_pool(name="actp", bufs=4))
        sc_p = ctx.enter_context(tc.tile_pool(name="scp", bufs=2))
        mr_p = ctx.enter_context(tc.tile_pool(name="mrp", bufs=3))
        sm_p = ctx.enter_context(tc.tile_pool(name="smp", bufs=3))
        att_p = ctx.enter_context(tc.tile_pool(name="attp", bufs=2))
        attnT_p = ctx.enter_context(tc.tile_pool(name="attnTp", bufs=8))
        ps_sc = ctx.enter_context(tc.tile_pool(name="ps_sc", bufs=2, space="PSUM"))
        ps_aux = ctx.enter_context(tc.tile_pool(name="ps_aux", bufs=2, space="PSUM"))
        ps_tr = ctx.enter_context(tc.tile_pool(name="ps_tr", bufs=2, space="PSUM"))
        ps_out = ctx.enter_context(tc.tile_pool(name="ps_out", bufs=1, space="PSUM"))

        # ---------- constants ----------
        ident = const_p.tile([128, 128], F32, tag="ident")
        nc.vector.memset(ident[:], 1.0)
        nc.gpsimd.affine_select(ident[:], ident[:], [[-1, 128]],
                                AL.is_equal, 0.0, base=0, channel_multiplier=1)
        ktile = const_p.tile([128, TOPK], F32, tag="ktile")
        nc.sync.dma_start(ktile[:], kt_d[:])
        asm = const_p.tile([U, H], F32, tag="asm")
        nc.sync.dma_start(asm[:], asm_d[:])
        ablock = const_p.tile([128, H * 256], F32, tag="ablock")
        for h in range(H):
            nc.sync.dma_start(ablock[:, h * 256:(h + 1) * 256], abl_d[h])
        zeros32 = const_p.tile([128, TOPK], F32, tag="zeros32")
        nc.vector.memset(zeros32[:], 0)
        eps9 = const_p.tile([128, 1], F32, tag="eps9")
        nc.vector.memset(eps9[:], 1e-9)
        onesrow = const_p.tile([1, 128], F32, tag="onesrow")
        nc.vector.memset(onesrow[:], 1.0)

        # ---------- features -> fT [128 d, 512 n] ----------
        feat_sb = setup_p.tile([128, 512], F32, tag="feat")
        nc.sync.dma_start(
            feat_sb[:].rearrange("p (c d) -> p c d", c=4),
            feat_d.rearrange("(c p) d -> p c d", p=128))
        fT = const_p.tile([128, 512], F32, tag="fT")
        for ib in range(4):
            ptr = ps_tr.tile([128, 128], F32, tag="tr")
            nc.tensor.transpose(ptr[:], feat_sb[:, 128 * ib:128 * (ib + 1)], ident[:])
            nc.vector.tensor_copy(fT[:, 128 * ib:128 * (ib + 1)], ptr[:])

        # ---------- adjacency -> additive mask madd[ib] ----------
        madds = []
        for ib in range(4):
            adj_i = setup_p.tile([128, 512], I32, tag="adji")
            nc.sync.dma_start(adj_i[:], adj_d[128 * ib:128 * (ib + 1), :])
            madd = madd_p.tile([128, 512], F32, tag="madd")
            nc.vector.tensor_copy(madd[:], adj_i[:])
            # self loop on the diagonal chunk
            nc.vector.tensor_tensor(
                madd[:, 128 * ib:128 * (ib + 1)],
                madd[:, 128 * ib:128 * (ib + 1)], ident[:], AL.add)
            # madd = -1e9 * relu(1 - c) ; c>=1 -> 0, c==0 -> -1e9
            nc.vector.tensor_scalar(madd[:], madd[:], -1.0, 1.0, AL.mult, AL.add)
            nc.vector.tensor_scalar(madd[:], madd[:], 0.0, NEG, AL.max, AL.mult)
            madds.append(madd)

        # persistent output accumulator (attn @ S)^T summed over heads
        poutT = ps_out.tile([64, 512], F32, tag="poutT")

        for h in range(H):
            # ---------- head setup ----------
            wsrc = head_p.tile([128, U], F32, tag="wsrc")
            nc.sync.dma_start(wsrc[:], wsrc_d[h])
            wdst = head_p.tile([128, U], F32, tag="wdst")
            nc.sync.dma_start(wdst[:], wdst_d[h])

            # srcT / dstT [64, 512]
            p_src = ps_aux.tile([64, 512], F32, tag="aux")
            nc.tensor.matmul(p_src[:], wsrc[:], fT[:], start=True, stop=True)
            srcT2s = head_p.tile([128, 512], F32, tag="srcT2s")
            nc.vector.tensor_copy(srcT2s[0:64, :], p_src[:])
            nc.vector.tensor_copy(srcT2s[64:128, 0:511], p_src[:, 1:512])

            p_dst = ps_aux.tile([64, 512], F32, tag="aux")
            nc.tensor.matmul(p_dst[:], wdst[:], fT[:], start=True, stop=True)
            dstT2 = head_p.tile([128, 512], F32, tag="dstT2")
            nc.vector.tensor_copy(dstT2[0:64, :], p_dst[:])
            nc.vector.tensor_copy(dstT2[64:128, :], p_dst[:])

            # S natural [128, 4*64]
            Ssb = head_p.tile([128, 4 * U], F32, tag="Ssb")
            for jc in range(4):
                p_s = ps_aux.tile([128, U], F32, tag="aux")
                nc.tensor.matmul(p_s[:], fT[:, 128 * jc:128 * (jc + 1)], wsrc[:],
                                 start=True, stop=True)
                nc.vector.tensor_copy(Ssb[:, U * jc:U * (jc + 1)], p_s[:])

            # sdot columns (0.025 * S @ a) [128, 4]
            sdot = head_p.tile([128, 4], F32, tag="sdot")
            for ib in range(4):
                p_sd = ps_aux.tile([128, 1], F32, tag="aux")
                nc.tensor.matmul(p_sd[:], srcT2s[0:64, 128 * ib:128 * (ib + 1)],
                                 asm[:, h:h + 1], start=True, stop=True)
                nc.vector.tensor_copy(sdot[:, ib:ib + 1], p_sd[:])

            # ddot row -> broadcast [128, 512] (kept in PSUM)
            p_dd = ps_aux.tile([1, 512], F32, tag="aux")
            nc.tensor.matmul(p_dd[:], asm[:, h:h + 1], dstT2[0:64, :],
                             start=True, stop=True)
            ddrow = head_p.tile([1, 512], F32, tag="ddrow")
            nc.vector.tensor_copy(ddrow[:], p_dd[:])
            p_ddbc = ps_aux.tile([128, 512], F32, tag="aux")
            nc.tensor.matmul(p_ddbc[:], onesrow[:], ddrow[:], start=True, stop=True)

            # M2[ib] = madd + sdot_col + ddot_row
            m2s = []
            for ib in range(4):
                m2 = m2_p.tile([128, 512], F32, tag="m2")
                nc.vector.scalar_tensor_tensor(m2[:], madds[ib][:], sdot[:, ib:ib + 1],
                                               p_ddbc[:], AL.add, AL.add)
                m2s.append(m2)

            attnTs = [attnT_p.tile([128, 512], F32, tag="attnT", name=f"attnT{h}_{jc}")
                      for jc in range(4)]

            abl_h = ablock[:, h * 256:(h + 1) * 256]
            for ib in range(4):
                # ---------- relu pass + contraction ----------
                psc = ps_sc.tile([128, 512], F32, tag="psc")
                for t in range(64):
                    act_t = act_p.tile([128, 512], F32, tag="actt")
                    bias_col = srcT2s[:, 128 * ib + 2 * t:128 * ib + 2 * t + 1]
                    nc.scalar.activation(act_t[:], dstT2[:], AF.Relu, bias=bias_col)
                    nc.tensor.matmul(psc[:], abl_h[:, 126 - 2 * t:254 - 2 * t],
                                     act_t[:], start=(t == 0), stop=(t == 63))

                # sc = psc + M2
                sc = sc_p.tile([128, 512], F32, tag="sc")
                nc.vector.scalar_tensor_tensor(sc[:], psc[:], 1.0, m2s[ib][:],
                                               AL.mult, AL.add)

                # ---------- top-32 (sorted desc) ----------
                tk = sm_p.tile([128, TOPK], F32, tag="tk")
                nc.vector.max(tk[:, 0:8], sc[:])
                prev = sc
                for r in range(1, 4):
                    mr = mr_p.tile([128, 512], F32, tag="mr")
                    nc.vector.match_replace(mr[:], tk[:, 8 * (r - 1):8 * r], prev[:], REPL)
                    nc.vector.max(tk[:, 8 * r:8 * (r + 1)], mr[:])
                    prev = mr

                # ---------- sparsemax on the sorted 32-block ----------
                sum32 = sm_p.tile([128, 1], F32, tag="sum32")
                nc.vector.tensor_reduce(sum32[:], tk[:], AX.X, AL.add)
                vm = sm_p.tile([128, 1], F32, tag="vm")
                nc.vector.tensor_scalar(vm[:], sum32[:], 1.0 / 16.0, None, AL.mult)
                z = sm_p.tile([128, TOPK], F32, tag="z")
                nc.vector.tensor_scalar(z[:], tk[:], 2.0, vm[:], AL.mult, AL.subtract)
                cs = sm_p.tile([128, TOPK], F32, tag="cs")
                nc.vector.tensor_tensor_scan(cs[:], z[:], zeros32[:], 0.0, AL.add, AL.add)
                w = sm_p.tile([128, TOPK], F32, tag="w")
                nc.vector.tensor_tensor(w[:], z[:], ktile[:], AL.mult)
                sup = sm_p.tile([128, TOPK], F32, tag="sup")
                nc.vector.scalar_tensor_tensor(sup[:], w[:], 1.0, cs[:], AL.add, AL.is_gt)
                kstar = sm_p.tile([128, 1], F32, tag="kstar")
                nc.vector.tensor_reduce(kstar[:], sup[:], AX.X, AL.add)
                zs = sm_p.tile([128, TOPK], F32, tag="zs")
                nc.vector.tensor_tensor(zs[:], z[:], sup[:], AL.mult)
                zssum = sm_p.tile([128, 1], F32, tag="zssum")
                nc.vector.tensor_reduce(zssum[:], zs[:], AX.X, AL.add)
                rk = sm_p.tile([128, 1], F32, tag="rk")
                nc.vector.reciprocal(rk[:], kstar[:])
                tau = sm_p.tile([128, 1], F32, tag="tau")
                nc.vector.tensor_scalar(tau[:], zssum[:], 1.0, rk[:], AL.subtract, AL.mult)
                nb = sm_p.tile([128, 1], F32, tag="nb")
                nc.vector.tensor_scalar(nb[:], tau[:], vm[:], -1.0, AL.add, AL.mult)

                # ---------- attn = relu(2*sc_masked - (vm+tau)), sharpen, norm ----------
                lt = mr_p.tile([128, 512], F32, tag="mr")
                nc.vector.tensor_scalar(lt[:], sc[:], tk[:, 31:32], None, AL.is_lt)
                pre = mr_p.tile([128, 512], F32, tag="mr")
                nc.vector.scalar_tensor_tensor(pre[:], lt[:], NEG, sc[:], AL.mult, AL.add)
                attn = att_p.tile([128, 512], F32, tag="attn")
                nc.scalar.activation(attn[:], pre[:], AF.Relu, bias=nb[:], scale=2.0)
                rs = sm_p.tile([128, 1], F32, tag="rs")
                sq = att_p.tile([128, 512], F32, tag="sq")
                nc.scalar.activation(sq[:], attn[:], AF.Square, bias=eps9[:],
                                     accum_out=rs[:])
                dn = sm_p.tile([128, 1], F32, tag="dn")
                nc.vector.tensor_scalar(dn[:], rs[:], 1e-9, 4.0, AL.add, AL.mult)
                rden = sm_p.tile([128, 1], F32, tag="rden")
                nc.vector.reciprocal(rden[:], dn[:])
                attn_n = att_p.tile([128, 512], F32, tag="attn_n")
                nc.vector.tensor_scalar(attn_n[:], sq[:], rden[:], None, AL.mult)

                # ---------- transpose attn to [j, i] blocks ----------
                for jc in range(4):
                    ptr = ps_tr.tile([128, 128], F32, tag="tr")
                    nc.tensor.transpose(ptr[:], attn_n[:, 128 * jc:128 * (jc + 1)],
                                        ident[:])
                    nc.vector.tensor_copy(attnTs[jc][:, 128 * ib:128 * (ib + 1)], ptr[:])

            # ---------- value matmuls: poutT += S^T-chunks @ attnT ----------
            for jc in range(4):
                nc.tensor.matmul(poutT[:], Ssb[:, U * jc:U * (jc + 1)], attnTs[jc][:],
                                 start=(h == 0 and jc == 0), stop=(h == H - 1 and jc == 3))

        # ---------- final transpose + store ----------
        outT_sb = setup_p.tile([64, 512], F32, tag="outT")
        nc.vector.tensor_copy(outT_sb[:], poutT[:])
        out_sb = setup_p.tile([128, 4 * U], F32, tag="outsb")
        for ib in range(4):
            ptr = ps_tr.tile([128, U], F32, tag="tr")
            nc.tensor.transpose(ptr[:], outT_sb[:, 128 * ib:128 * (ib + 1)],
                                ident[0:64, 0:64])
            nc.vector.tensor_copy(out_sb[:, U * ib:U * (ib + 1)], ptr[:])
        nc.sync.dma_start(
            out_d.rearrange("(c p) u -> p c u", p=128),
            out_sb[:].rearrange("p (c u) -> p c u", c=4))

    nc.compile()
    return nc


def _host_consts(W_src, W_dst, a):
    a2 = a.reshape(H, U).astype(np.float32)
    ablock = np.zeros((H, 128, 256), np.float32)
    for h in range(H):
        ablock[h, 0:64, 126] = 0.1 * a2[h]
        ablock[h, 64:128, 127] = 0.1 * a2[h]
    asm = (0.025 * a2.T).astype(np.float32).copy()          # [64, 4]
    ktile = np.broadcast_to(np.arange(1, TOPK + 1, dtype=np.float32),
                            (128, TOPK)).copy()
    return ablock, asm, ktile


def kernel(features, adjacency, W_src, W_dst, a):
    if "nc" not in _cache:
        _cache["nc"] = _build_program()
    nc = _cache["nc"]
    ablock, asm, ktile = _host_consts(W_src, W_dst, a)
    wsrc = np.ascontiguousarray(W_src, dtype=np.float32)
    wdst = np.ascontiguousarray(W_dst, dtype=np.float32)
    in_maps = []
    for b in range(B):
        in_maps.append({
            "feat": np.ascontiguousarray(features[b], dtype=np.float32),
            "adj": np.ascontiguousarray(adjacency[b], dtype=np.int32),
            "wsrc": wsrc, "wdst": wdst,
            "ablock": ablock, "asm": asm, "ktile": ktile,
        })
    res = run_bass_kernel_spmd(nc, in_maps, list(range(B)))
    out = np.stack([res.results[b]["out"] for b in range(B)], axis=0)
    return out.astype(np.float32)
